# revision 7
# baseline (speedup 1.0000x reference)
"""Encoder layer (pre-norm attention + MLP) on 8 Trainium2 cores.

Sharding: core = (batch b in 0..3, half hf in 0..1). Each core receives the
full 2048-token sequence of batch b, transposed to [E, S] and rolled so the
core's own 1024 tokens are columns 0:1024 (attention and LN are invariant to
key order, so rolling keeps the program identical across cores). The core
computes K/V over the full sequence and everything else only for its own
tokens. No collectives; the host reassembles the 8 shards.

v2: everything stays in SBUF (K/Q/V, the MLP hidden h) — no DRAM round
trips. fc1/fc2 weights and activations are bf16 (halves weight DMA).
Attention scores are packed 4 MMs per PE pass via base-partition tile
placement (head pairs live at partitions 0:64 / 64:128), recovering the
full 128x128 array despite D=64. QKV projection matmuls are emitted
interleaved with attention so the PE has filler work while ScalarE runs
the softmax exponentials. The final output fuses fc2 + b_fc2 + attention
residual on-chip; the host only transposes.
"""

import numpy as np
import ml_dtypes
from contextlib import ExitStack

import concourse.bacc as bacc
import concourse.mybir as mybir
import concourse.tile as tile
from concourse.bass_utils import run_bass_kernel_spmd

F32 = mybir.dt.float32
F32R = mybir.dt.float32r
BF16 = mybir.dt.bfloat16
AF = mybir.ActivationFunctionType
OP = mybir.AluOpType

B, S, E, H, D, FF = 4, 2048, 1024, 16, 64, 4096
TOWN = 1024  # tokens owned per core
ET = E // 128  # 8
FT = FF // 128  # 32
NT = S // 128  # 16 token tiles (full seq)
NP = H // 2  # 8 head pairs
NCORES = 8
EPS = 1e-6


def _build():
    nc = bacc.Bacc()

    x_t = nc.dram_tensor("x_t", [E, S], F32R, kind="ExternalInput")
    # weights pre-tiled on host: [out_tile, 128(part=e%128), e_tile, out_in_tile]
    wq_t = nc.dram_tensor("wq_t", [ET, 128, ET, 128], BF16,
                          kind="ExternalInput")
    wk_t = nc.dram_tensor("wk_t", [ET, 128, ET, 128], BF16,
                          kind="ExternalInput")
    wv_t = nc.dram_tensor("wv_t", [2, 128, ET, 512], BF16,
                          kind="ExternalInput")
    qb = nc.dram_tensor("qb", [128, ET], F32, kind="ExternalInput")
    kb = nc.dram_tensor("kb", [128, ET], F32, kind="ExternalInput")
    vb = nc.dram_tensor("vb", [E], BF16, kind="ExternalInput")
    wout_t = nc.dram_tensor("wout_t", [ET, 128, ET, 128], BF16,
                            kind="ExternalInput")
    ob = nc.dram_tensor("ob", [128, ET], F32, kind="ExternalInput")
    wfc1_t = nc.dram_tensor("wfc1_t", [FT, 128, ET, 128], BF16,
                            kind="ExternalInput")
    f1b = nc.dram_tensor("f1b", [128, FT], F32, kind="ExternalInput")
    wfc2_t = nc.dram_tensor("wfc2_t", [ET, 128, FT, 128], BF16,
                            kind="ExternalInput")
    f2b = nc.dram_tensor("f2b", [128, ET], F32, kind="ExternalInput")

    out_t = nc.dram_tensor("out_t", [E, TOWN], F32, kind="ExternalOutput")

    inv_e = 1.0 / E
    unb = float(E) / (E - 1.0)  # E/(E-1) for unbiased variance

    with tile.TileContext(nc) as tc, ExitStack() as ctx:
        consts = ctx.enter_context(tc.tile_pool(name="consts", bufs=1))
        ones_f32 = consts.tile([128, 256], F32)
        nc.vector.memset(ones_f32, 1.0)
        ones128 = consts.tile([128, 128], F32R)
        nc.vector.tensor_copy(ones128, ones_f32[:, 0:128])
        qb_sb = consts.tile([128, ET], F32)
        kb_sb = consts.tile([128, ET], F32)
        ob_sb = consts.tile([128, ET], F32)
        f1b_sb = consts.tile([128, FT], F32)
        f2b_sb = consts.tile([128, ET], F32)
        nc.sync.dma_start(out=qb_sb, in_=qb[:, :])
        nc.sync.dma_start(out=kb_sb, in_=kb[:, :])
        nc.sync.dma_start(out=ob_sb, in_=ob[:, :])
        nc.sync.dma_start(out=f1b_sb, in_=f1b[:, :])
        nc.sync.dma_start(out=f2b_sb, in_=f2b[:, :])
        ones_bf16 = consts.tile([1, 128], BF16)
        nc.vector.tensor_copy(ones_bf16, ones_f32[0:1, 0:128])
        # v bias broadcast across all partitions (v is token-major)
        vb_row = consts.tile([1, E], BF16)
        nc.sync.dma_start(out=vb_row, in_=vb[None, :])
        vb_bc = consts.tile([128, E], BF16)
        with tc.tile_pool(name="vbbc_p", bufs=2, space="PSUM") as vbbc_p:
            for c in range(2):
                ps = vbbc_p.tile([128, 512], F32, tag="vbbc")
                nc.tensor.matmul(
                    ps, ones_bf16,
                    vb_row[:, c * 512:(c + 1) * 512],
                    start=True, stop=True,
                )
                nc.vector.tensor_copy(vb_bc[:, c * 512:(c + 1) * 512], ps)

        # Weight pools live in the outer scope (never released): tiles
        # written by input DMA must not reuse released pool space (the DMA
        # has no happens-before chain to prior readers). All slots are
        # materialized up front by prefetching the first weight tiles.
        pb_wv = ctx.enter_context(tc.tile_pool(name="pb_wv", bufs=1))
        pb_w = ctx.enter_context(tc.tile_pool(name="pb_w", bufs=2))
        pw = ctx.enter_context(tc.tile_pool(name="pw", bufs=2))

        def load_wv(cv):
            wv_c = pb_wv.tile([128, ET, 512], BF16, tag="wv")
            nc.sync.dma_start(out=wv_c, in_=wv_t[cv])
            return wv_c

        def load_wkq(p):
            wk_p = pb_w.tile([128, ET, 128], BF16, tag="wk")
            nc.sync.dma_start(out=wk_p, in_=wk_t[p])
            wq_p = pb_w.tile([128, ET, 128], BF16, tag="wk")
            nc.sync.dma_start(out=wq_p, in_=wq_t[p])
            return wk_p, wq_p

        def load_w(src, ot, n_in):
            w = pw.tile([128, n_in, 128], BF16, tag="w")
            nc.sync.dma_start(out=w, in_=src[ot])
            return w

        # Prefetch: fixes weight-pool slot addresses before any pool is
        # released, and hides the first loads under stage A.
        wv0 = load_wv(0)
        wkq0 = load_wkq(0)
        wd0 = load_w(wout_t, 0, ET)
        wd1 = load_w(wout_t, 1, ET)

        # Long-lived cross-stage tensors. Pools are opened/closed in LIFO
        # (stack) order; x2 reuses the x_own buffer in place after D.
        s_xown = ExitStack()   # x_own: A..D, then reused as x2: D..G
        s_ctxn = ExitStack()   # ctxn: C..D
        s_z1 = ExitStack()     # z1: A..C (needed through B emissions)
        s_bc = ExitStack()     # k/q/v + B/C working pools: B..C
        s_x = ExitStack()      # x_sb: A only
        s_z2 = ExitStack()     # z2: E..F
        s_h = ExitStack()      # h: F..G

        p_xown = s_xown.enter_context(tc.tile_pool(name="p_xown", bufs=1))
        x_own = p_xown.tile([128, ET, TOWN], F32R)
        p_ctxn = s_ctxn.enter_context(tc.tile_pool(name="p_ctxn", bufs=1))
        ctxn = p_ctxn.tile([128, NP, TOWN], BF16)
        p_z1 = s_z1.enter_context(tc.tile_pool(name="p_z1", bufs=1))
        z1 = p_z1.tile([128, ET, S], BF16)
        p_x = s_x.enter_context(tc.tile_pool(name="p_x", bufs=1))
        x_sb = p_x.tile([128, ET, S], F32R)

        # ---------------- Stage A: LN1 stats + z1 over full sequence -------
        xre = x_t.rearrange("(a p) s -> p a s", p=128)
        CA = 512
        with tc.tile_pool(name="pa_st", bufs=1) as pa_st, \
             tc.tile_pool(name="pa_xsq", bufs=2) as pa_xsq, \
             tc.tile_pool(name="pa_misc", bufs=1) as pa_misc, \
             tc.tile_pool(name="pa_ps", bufs=2, space="PSUM") as pa_ps:
            mean1 = pa_st.tile([128, S], F32)
            rstd1 = pa_st.tile([128, S], F32)
            for c in range(S // CA):
                sl = slice(c * CA, (c + 1) * CA)
                nc.sync.dma_start(out=x_sb[:, :, sl], in_=xre[:, :, sl])
                ps_sum = pa_ps.tile([128, CA], F32, tag="ps_sum")
                ps_ssq = pa_ps.tile([128, CA], F32, tag="ps_ssq")
                for a in range(ET):
                    xa = x_sb[:, a, sl]
                    xsq = pa_xsq.tile([128, CA], F32R, tag="xsq")
                    nc.vector.tensor_tensor(xsq, xa, xa, OP.mult)
                    nc.tensor.matmul(ps_sum, ones128, xa,
                                     start=(a == 0), stop=(a == ET - 1))
                    nc.tensor.matmul(ps_ssq, ones128, xsq,
                                     start=(a == 0), stop=(a == ET - 1))
                nc.vector.tensor_scalar_mul(mean1[:, sl], ps_sum, inv_e)
                msq = pa_misc.tile([128, CA], F32, tag="msq")
                nc.vector.scalar_tensor_tensor(
                    msq, mean1[:, sl], unb, mean1[:, sl], OP.mult, OP.mult)
                var = pa_misc.tile([128, CA], F32, tag="var")
                nc.vector.scalar_tensor_tensor(
                    var, ps_ssq, 1.0 / (E - 1.0), msq, OP.mult,
                    OP.subtract)
                std = pa_misc.tile([128, CA], F32, tag="std")
                nc.scalar.activation(std, var, AF.Sqrt)
                nc.vector.tensor_scalar_add(std, std, EPS)
                nc.vector.reciprocal(rstd1[:, sl], std)
                for a in range(ET):
                    nc.vector.tensor_tensor(
                        z1[:, a, sl], x_sb[:, a, sl], mean1[:, sl],
                        OP.subtract)
                    nc.vector.tensor_tensor(
                        z1[:, a, sl], z1[:, a, sl], rstd1[:, sl], OP.mult)
                if c < 2:
                    nc.vector.tensor_copy(x_own[:, :, sl], x_sb[:, :, sl])
        s_x.close()  # x_sb dead after A

        # ------------- Stages B+C interleaved ------------------------------
        # B(p): K/Q projections for head pair p (K over full seq, Q own),
        # V in two 4-pair chunks. C(p): attention for pair p. B(p+2) is
        # emitted before C(p) so the PE has projection matmuls to run while
        # ScalarE computes exp() for pair p.
        p_kqv = s_bc.enter_context(tc.tile_pool(name="p_kqv", bufs=1))
        k_sb = p_kqv.tile([128, NP, S], BF16)
        q_sb = p_kqv.tile([128, NP, TOWN], BF16)
        # [part = t%128, t_tile, pair, head-in-pair, 64 v dims + 1 ones col]
        v_sb = p_kqv.tile([128, NT, NP, 2, 65], BF16)
        nc.vector.tensor_copy(
            v_sb[:, :, :, :, 64],
            ones_f32[:, 0:NT * NP * 2].rearrange(
                "p (a b c) -> p a b c", a=NT, b=NP))
        pb_psv = s_bc.enter_context(
            tc.tile_pool(name="pb_psv", bufs=1, space="PSUM"))
        pb_pskq = s_bc.enter_context(
            tc.tile_pool(name="pb_pskq", bufs=1, space="PSUM"))

        def b_v(cv, wv_c=None):
            if wv_c is None:
                wv_c = load_wv(cv)
            for tt in range(NT):
                tsl = slice(tt * 128, (tt + 1) * 128)
                ps = pb_psv.tile([128, 512], F32, tag="psv")
                for a in range(ET):
                    nc.tensor.matmul(ps, z1[:, a, tsl], wv_c[:, a, :],
                                     start=(a == 0), stop=(a == ET - 1))
                nc.vector.tensor_tensor(
                    v_sb[:, tt, 4 * cv:4 * (cv + 1), :, 0:64],
                    ps.rearrange("p (g hh w) -> p g hh w", g=4, hh=2),
                    vb_bc[:, cv * 512:(cv + 1) * 512].rearrange(
                        "p (g hh w) -> p g hh w", g=4, hh=2),
                    OP.add)

        def b_kq(p, wkq=None):
            wk_p, wq_p = wkq if wkq is not None else load_wkq(p)
            for c in range(4):
                csl = slice(c * 512, (c + 1) * 512)
                ps = pb_pskq.tile([128, 512], F32, tag="pskq")
                for a in range(ET):
                    nc.tensor.matmul(ps, wk_p[:, a, :], z1[:, a, csl],
                                     start=(a == 0), stop=(a == ET - 1))
                nc.vector.tensor_scalar_add(k_sb[:, p, csl], ps,
                                            kb_sb[:, p:p + 1])
            for c in range(2):
                csl = slice(c * 512, (c + 1) * 512)
                ps = pb_pskq.tile([128, 512], F32, tag="pskq")
                for a in range(ET):
                    nc.tensor.matmul(ps, wq_p[:, a, :], z1[:, a, csl],
                                     start=(a == 0), stop=(a == ET - 1))
                nc.vector.tensor_scalar_add(q_sb[:, p, csl], ps,
                                            qb_sb[:, p:p + 1])

        pc_s = s_bc.enter_context(
            tc.tile_pool(name="pc_s", bufs=1, space="PSUM"))
        pc_ctx = s_bc.enter_context(
            tc.tile_pool(name="pc_ctx", bufs=1, space="PSUM"))
        pc_rb = s_bc.enter_context(
            tc.tile_pool(name="pc_rb", bufs=1, space="PSUM"))
        pc_pr = s_bc.enter_context(tc.tile_pool(name="pc_pr", bufs=2))
        pc_m = s_bc.enter_context(tc.tile_pool(name="pc_m", bufs=1))

        def c_attn(p):
            for qc in range(2):
                qsl = slice(qc * 512, (qc + 1) * 512)
                ctx_h = pc_ctx.tile([65, 512], F32, tag="ctxa")
                ctx_h2 = pc_ctx.tile([65, 512], F32, tag="ctxb")
                for kt in range(NT):
                    k0 = kt * 128
                    s_ps = pc_s.tile([128, 2, 512], F32, tag="s")
                    # 4 packed score MMs: head pair at partitions 0:64 /
                    # 64:128 of k/q, k-token halves at psum 0:64 / 64:128.
                    nc.tensor.matmul(
                        s_ps[0:64, 0, :], k_sb[0:64, p, k0:k0 + 64],
                        q_sb[0:64, p, qsl], start=True, stop=True)
                    nc.tensor.matmul(
                        s_ps[64:128, 0, :], k_sb[0:64, p, k0 + 64:k0 + 128],
                        q_sb[0:64, p, qsl], start=True, stop=True)
                    nc.tensor.matmul(
                        s_ps[0:64, 1, :], k_sb[64:128, p, k0:k0 + 64],
                        q_sb[64:128, p, qsl], start=True, stop=True)
                    nc.tensor.matmul(
                        s_ps[64:128, 1, :],
                        k_sb[64:128, p, k0 + 64:k0 + 128],
                        q_sb[64:128, p, qsl], start=True, stop=True)
                    pr = pc_pr.tile([128, 2, 512], BF16, tag="pr")
                    nc.scalar.activation(pr, s_ps, AF.Exp, scale=0.125)
                    nc.tensor.matmul(ctx_h, v_sb[:, kt, p, 0, :],
                                     pr[:, 0, :],
                                     start=(kt == 0), stop=(kt == NT - 1))
                    nc.tensor.matmul(ctx_h2, v_sb[:, kt, p, 1, :],
                                     pr[:, 1, :],
                                     start=(kt == 0), stop=(kt == NT - 1))
                rec = pc_m.tile([1, 2, 512], BF16, tag="rec")
                with nc.allow_low_precision(
                        reason="bf16 rounding of softmax denom"):
                    nc.vector.reciprocal(rec[:, 0, :], ctx_h[64:65, :])
                    nc.vector.reciprocal(rec[:, 1, :], ctx_h2[64:65, :])
                rb = pc_rb.tile([64, 2, 512], F32, tag="rb")
                nc.tensor.matmul(rb[:, 0, :], ones_bf16[:, 0:64],
                                 rec[:, 0, :], start=True, stop=True)
                nc.tensor.matmul(rb[:, 1, :], ones_bf16[:, 0:64],
                                 rec[:, 1, :], start=True, stop=True)
                rb_sb = pc_m.tile([64, 2, 512], BF16, tag="rbs")
                nc.vector.tensor_copy(rb_sb, rb)
                nc.vector.tensor_tensor(
                    ctxn[0:64, p, qsl], ctx_h[0:64, :], rb_sb[:, 0, :],
                    OP.mult)
                nc.vector.tensor_tensor(
                    ctxn[64:128, p, qsl], ctx_h2[0:64, :], rb_sb[:, 1, :],
                    OP.mult)

        b_v(0, wv0)
        b_kq(0, wkq0)
        b_kq(1)
        for p in range(NP):
            if p == 2:
                b_v(1)
            if p + 2 < NP:
                b_kq(p + 2)
            c_attn(p)
        s_bc.close()
        s_z1.close()

        # ------------- Stage D: out-proj + residual ------------------------
        # x2 overwrites x_own in place (the residual add reads x_own[ot,csl]
        # and writes the same region in one DVE instruction).
        x2 = x_own
        with tc.tile_pool(name="pd_ps", bufs=2, space="PSUM") as pdp:
            for ot in range(ET):
                if ot == 0:
                    w_ot = wd0
                elif ot == 1:
                    w_ot = wd1
                else:
                    w_ot = load_w(wout_t, ot, ET)
                for c in range(2):
                    csl = slice(c * 512, (c + 1) * 512)
                    ps = pdp.tile([128, 512], F32, tag="ps")
                    for a in range(ET):
                        nc.tensor.matmul(ps, w_ot[:, a, :],
                                         ctxn[:, a, csl],
                                         start=(a == 0), stop=(a == ET - 1))
                    nc.vector.scalar_tensor_tensor(
                        x2[:, ot, csl], ps, ob_sb[:, ot:ot + 1],
                        x_own[:, ot, csl], OP.add, OP.add)
        s_ctxn.close()

        # --------- Stage E: LN2 stats + z2 (own tokens) --------------------
        p_h = s_h.enter_context(tc.tile_pool(name="p_h", bufs=1))
        h_sb = p_h.tile([128, FT, TOWN], BF16)
        p_z2 = s_z2.enter_context(tc.tile_pool(name="p_z2", bufs=1))
        z2 = p_z2.tile([128, ET, TOWN], BF16)
        with tc.tile_pool(name="pe_st", bufs=1) as pe_st, \
             tc.tile_pool(name="pe_tmp", bufs=2) as pe_tmp, \
             tc.tile_pool(name="pe_ps", bufs=2, space="PSUM") as pe_ps:
            mean2 = pe_st.tile([128, TOWN], F32)
            rstd2 = pe_st.tile([128, TOWN], F32)
            for c in range(2):
                sl = slice(c * 512, (c + 1) * 512)
                ps_sum = pe_ps.tile([128, 512], F32, tag="ps_sum")
                ps_ssq = pe_ps.tile([128, 512], F32, tag="ps_ssq")
                for a in range(ET):
                    xa = x2[:, a, sl]
                    xsq = pe_tmp.tile([128, 512], F32R, tag="xsq")
                    nc.vector.tensor_tensor(xsq, xa, xa, OP.mult)
                    nc.tensor.matmul(ps_sum, ones128, xa,
                                     start=(a == 0), stop=(a == ET - 1))
                    nc.tensor.matmul(ps_ssq, ones128, xsq,
                                     start=(a == 0), stop=(a == ET - 1))
                nc.vector.tensor_scalar_mul(mean2[:, sl], ps_sum, inv_e)
                msq = pe_tmp.tile([128, 512], F32, tag="msq")
                nc.vector.scalar_tensor_tensor(
                    msq, mean2[:, sl], unb, mean2[:, sl], OP.mult, OP.mult)
                var = pe_tmp.tile([128, 512], F32, tag="var")
                nc.vector.scalar_tensor_tensor(
                    var, ps_ssq, 1.0 / (E - 1.0), msq, OP.mult, OP.subtract)
                std = pe_tmp.tile([128, 512], F32, tag="std")
                nc.scalar.activation(std, var, AF.Sqrt)
                nc.vector.tensor_scalar_add(std, std, EPS)
                nc.vector.reciprocal(rstd2[:, sl], std)
                for a in range(ET):
                    nc.vector.tensor_tensor(
                        z2[:, a, sl], x2[:, a, sl], mean2[:, sl],
                        OP.subtract)
                    nc.vector.tensor_tensor(
                        z2[:, a, sl], z2[:, a, sl], rstd2[:, sl], OP.mult)

        # --------- Stage F: fc1 + gelu -> h (SBUF) -------------------------
        with tc.tile_pool(name="pf_ps", bufs=2, space="PSUM") as pfp:
            for ft in range(FT):
                w_ft = load_w(wfc1_t, ft, ET)
                for c in range(2):
                    csl = slice(c * 512, (c + 1) * 512)
                    ps = pfp.tile([128, 512], F32, tag="ps")
                    for a in range(ET):
                        nc.tensor.matmul(ps, w_ft[:, a, :], z2[:, a, csl],
                                         start=(a == 0), stop=(a == ET - 1))
                    nc.scalar.activation(h_sb[:, ft, csl], ps, AF.Gelu,
                                         bias=f1b_sb[:, ft:ft + 1])
        s_z2.close()

        # ---------------- Stage G: fc2 + residual -> out_t -----------------
        with tc.tile_pool(name="pg_o", bufs=3) as pgo, \
             tc.tile_pool(name="pg_ps", bufs=2, space="PSUM") as pgp:
            for ot in range(ET):
                w_ot = load_w(wfc2_t, ot, FT)
                for c in range(2):
                    csl = slice(c * 512, (c + 1) * 512)
                    ps = pgp.tile([128, 512], F32, tag="ps")
                    for f in range(FT):
                        nc.tensor.matmul(ps, w_ot[:, f, :], h_sb[:, f, csl],
                                         start=(f == 0), stop=(f == FT - 1))
                    osb = pgo.tile([128, 512], F32, tag="osb")
                    nc.vector.scalar_tensor_tensor(
                        osb, ps, f2b_sb[:, ot:ot + 1], x2[:, ot, csl],
                        OP.add, OP.add)
                    nc.sync.dma_start(
                        out=out_t[ot * 128:(ot + 1) * 128, csl], in_=osb)
        s_h.close()
        s_xown.close()

    nc.finalize()
    return nc


_NC_CACHE = {}


def _get_nc():
    if "nc" not in _NC_CACHE:
        _NC_CACHE["nc"] = _build()
    return _NC_CACHE["nc"]


def _tile_w(w_t, n_out_tiles, dtype=ml_dtypes.bfloat16):
    # [E_in, O] (in-feature rows) -> [O//128, 128, E_in//128, 128] so each
    # output-tile's weight block is contiguous (multi-KB runs per partition).
    e_in, o = w_t.shape
    arr = w_t.reshape(e_in // 128, 128, n_out_tiles, o // n_out_tiles)
    return np.ascontiguousarray(arr.transpose(2, 1, 0, 3).astype(dtype))


def _prepare_in_maps(inputs):
    f = np.float32
    x = np.asarray(inputs["x"], f)
    w_qkv = np.asarray(inputs["w_qkv"], np.float64)
    ln1_w = np.asarray(inputs["ln1_w"], np.float64)
    ln1_b = np.asarray(inputs["ln1_b"], np.float64)
    ln2_w = np.asarray(inputs["ln2_w"], np.float64)
    ln2_b = np.asarray(inputs["ln2_b"], np.float64)
    w_fc1 = np.asarray(inputs["w_fc1"], np.float64)

    wqkv_s = (w_qkv * ln1_w[None, :])  # fold LN1 gamma
    qkv_bias = ln1_b @ np.asarray(inputs["w_qkv"], np.float64).T  # [3E]
    wqkv_t = np.ascontiguousarray(wqkv_s.T, f)  # [E, 3E]
    wq_t = _tile_w(wqkv_t[:, 0:E], ET)
    wk_t = _tile_w(wqkv_t[:, E:2 * E], ET)
    wv_t = _tile_w(wqkv_t[:, 2 * E:3 * E], 2)  # [2,128,ET,512] rhs chunks
    col = lambda v: np.ascontiguousarray(
        np.asarray(v, f).reshape(-1, 128).T)  # [o] -> [128, o//128]
    qb = col(qkv_bias[0:E])
    kb = col(qkv_bias[E:2 * E])
    vb = np.ascontiguousarray(qkv_bias[2 * E:3 * E]).astype(ml_dtypes.bfloat16)

    wout_t = _tile_w(np.ascontiguousarray(np.asarray(inputs["w_out"], f).T),
                     ET)
    ob = col(inputs["b_out"])

    wfc1_s = (w_fc1 * ln2_w[None, :])
    f1b_flat = np.asarray(inputs["b_fc1"], np.float64) + ln2_b @ w_fc1.T
    f1b = col(f1b_flat)
    wfc1_t = _tile_w(np.ascontiguousarray(wfc1_s.T, f), FT)
    wfc2_t = _tile_w(np.ascontiguousarray(np.asarray(inputs["w_fc2"], f).T),
                     ET)
    f2b = col(inputs["b_fc2"])

    shared = dict(wq_t=wq_t, wk_t=wk_t, wv_t=wv_t, qb=qb, kb=kb, vb=vb,
                  wout_t=wout_t, ob=ob, wfc1_t=wfc1_t, f1b=f1b,
                  wfc2_t=wfc2_t, f2b=f2b)
    in_maps = []
    for core in range(NCORES):
        b, hf = divmod(core, 2)
        xs = np.roll(x[b], -hf * TOWN, axis=0)  # own tokens first
        x_tc = np.ascontiguousarray(xs.T)  # [E, S]
        in_maps.append(dict(x_t=x_tc, **shared))
    return in_maps


def _assemble(inputs, results):
    out = np.empty((B, S, E), np.float32)
    for core in range(NCORES):
        b, hf = divmod(core, 2)
        out[b, hf * TOWN:(hf + 1) * TOWN, :] = results[core]["out_t"].T
    return out


def run(inputs, **spmd_kwargs):
    nc = _get_nc()
    in_maps = _prepare_in_maps(inputs)
    res = run_bass_kernel_spmd(nc, in_maps, core_ids=list(range(NCORES)),
                               **spmd_kwargs)
    return _assemble(inputs, res.results), res


def kernel(**inputs):
    out, _ = run(inputs)
    return out


# revision 9
# speedup vs baseline: 1.2043x; 1.2043x over previous
"""Encoder layer (pre-norm attention + MLP) on 8 Trainium2 cores.

Sharding: core = (batch b in 0..3, half hf in 0..1). Each core receives the
full 2048-token sequence of batch b, transposed to [E, S] and rolled so the
core's own 1024 tokens are columns 0:1024 (attention and LN are invariant to
key order, so rolling keeps the program identical across cores). The core
computes K/V over the full sequence and everything else only for its own
tokens. No collectives; the host reassembles the 8 shards.

v2: everything stays in SBUF (K/Q/V, the MLP hidden h) — no DRAM round
trips. fc1/fc2 weights and activations are bf16 (halves weight DMA).
Attention scores are packed 4 MMs per PE pass via base-partition tile
placement (head pairs live at partitions 0:64 / 64:128), recovering the
full 128x128 array despite D=64. QKV projection matmuls are emitted
interleaved with attention so the PE has filler work while ScalarE runs
the softmax exponentials. The final output fuses fc2 + b_fc2 + attention
residual on-chip; the host only transposes.
"""

import numpy as np
import ml_dtypes
from contextlib import ExitStack

import concourse.bacc as bacc
import concourse.mybir as mybir
import concourse.tile as tile
from concourse.bass_utils import run_bass_kernel_spmd

F32 = mybir.dt.float32
F32R = mybir.dt.float32r
BF16 = mybir.dt.bfloat16
AF = mybir.ActivationFunctionType
OP = mybir.AluOpType

B, S, E, H, D, FF = 4, 2048, 1024, 16, 64, 4096
TOWN = 1024  # tokens owned per core
ET = E // 128  # 8
FT = FF // 128  # 32
NT = S // 128  # 16 token tiles (full seq)
NP = H // 2  # 8 head pairs
NCORES = 8
EPS = 1e-6


def _build():
    nc = bacc.Bacc()

    x_t = nc.dram_tensor("x_t", [E, S], F32R, kind="ExternalInput")
    # weights pre-tiled on host: [out_tile, 128(part=e%128), e_tile, out_in_tile]
    wq_t = nc.dram_tensor("wq_t", [ET, 128, ET, 128], BF16,
                          kind="ExternalInput")
    wk_t = nc.dram_tensor("wk_t", [ET, 128, ET, 128], BF16,
                          kind="ExternalInput")
    wv_t = nc.dram_tensor("wv_t", [2, 128, ET, 512], BF16,
                          kind="ExternalInput")
    qb = nc.dram_tensor("qb", [128, ET], F32, kind="ExternalInput")
    kb = nc.dram_tensor("kb", [128, ET], F32, kind="ExternalInput")
    vb = nc.dram_tensor("vb", [E], BF16, kind="ExternalInput")
    wout_t = nc.dram_tensor("wout_t", [ET, 128, ET, 128], BF16,
                            kind="ExternalInput")
    ob = nc.dram_tensor("ob", [128, ET], F32, kind="ExternalInput")
    wfc1_t = nc.dram_tensor("wfc1_t", [FT, 128, ET, 128], BF16,
                            kind="ExternalInput")
    f1b = nc.dram_tensor("f1b", [128, FT], F32, kind="ExternalInput")
    wfc2_t = nc.dram_tensor("wfc2_t", [ET, 128, FT, 128], BF16,
                            kind="ExternalInput")
    f2b = nc.dram_tensor("f2b", [128, ET], F32, kind="ExternalInput")

    out_t = nc.dram_tensor("out_t", [E, TOWN], F32, kind="ExternalOutput")

    inv_e = 1.0 / E
    unb = float(E) / (E - 1.0)  # E/(E-1) for unbiased variance

    with tile.TileContext(nc) as tc, ExitStack() as ctx:
        consts = ctx.enter_context(tc.tile_pool(name="consts", bufs=1))
        ones_f32 = consts.tile([128, 256], F32)
        nc.vector.memset(ones_f32, 1.0)
        ones128 = consts.tile([128, 128], F32R)
        nc.vector.tensor_copy(ones128, ones_f32[:, 0:128])
        qb_sb = consts.tile([128, ET], F32)
        kb_sb = consts.tile([128, ET], F32)
        ob_sb = consts.tile([128, ET], F32)
        f1b_sb = consts.tile([128, FT], F32)
        f2b_sb = consts.tile([128, ET], F32)
        nc.sync.dma_start(out=qb_sb, in_=qb[:, :])
        nc.sync.dma_start(out=kb_sb, in_=kb[:, :])
        nc.sync.dma_start(out=ob_sb, in_=ob[:, :])
        nc.sync.dma_start(out=f1b_sb, in_=f1b[:, :])
        nc.sync.dma_start(out=f2b_sb, in_=f2b[:, :])
        ones_bf16 = consts.tile([1, 128], BF16)
        nc.vector.tensor_copy(ones_bf16, ones_f32[0:1, 0:128])
        # v bias broadcast across all partitions (v is token-major)
        vb_row = consts.tile([1, E], BF16)
        nc.sync.dma_start(out=vb_row, in_=vb[None, :])
        vb_bc = consts.tile([128, E], BF16)
        with tc.tile_pool(name="vbbc_p", bufs=2, space="PSUM") as vbbc_p:
            for c in range(2):
                ps = vbbc_p.tile([128, 512], F32, tag="vbbc")
                nc.tensor.matmul(
                    ps, ones_bf16,
                    vb_row[:, c * 512:(c + 1) * 512],
                    start=True, stop=True,
                )
                nc.vector.tensor_copy(vb_bc[:, c * 512:(c + 1) * 512], ps)

        # Weight pools live in the outer scope (never released): tiles
        # written by input DMA must not reuse released pool space (the DMA
        # has no happens-before chain to prior readers). All slots are
        # materialized up front by prefetching the first weight tiles.
        pb_wv = ctx.enter_context(tc.tile_pool(name="pb_wv", bufs=1))
        pb_w = ctx.enter_context(tc.tile_pool(name="pb_w", bufs=2))
        pw = ctx.enter_context(tc.tile_pool(name="pw", bufs=2))

        def load_wv(cv):
            wv_c = pb_wv.tile([128, ET, 512], BF16, tag="wv")
            nc.sync.dma_start(out=wv_c, in_=wv_t[cv])
            return wv_c

        def load_wkq(p):
            wk_p = pb_w.tile([128, ET, 128], BF16, tag="wk")
            nc.sync.dma_start(out=wk_p, in_=wk_t[p])
            wq_p = pb_w.tile([128, ET, 128], BF16, tag="wk")
            nc.sync.dma_start(out=wq_p, in_=wq_t[p])
            return wk_p, wq_p

        def load_w(src, ot, n_in):
            w = pw.tile([128, n_in, 128], BF16, tag="w")
            nc.sync.dma_start(out=w, in_=src[ot])
            return w


        # Long-lived cross-stage tensors. Pools are opened/closed in LIFO
        # (stack) order; x2 reuses the x_own buffer in place after D.
        s_xown = ExitStack()   # x_own: A..D, then reused as x2: D..G
        s_ctxn = ExitStack()   # ctxn: C..D
        s_z1 = ExitStack()     # z1: A..C (needed through B emissions)
        s_bc = ExitStack()     # k/q/v + B/C working pools: B..C
        s_x = ExitStack()      # x_sb: A only
        s_z2 = ExitStack()     # z2: E..F
        s_h = ExitStack()      # h: F..G

        p_xown = s_xown.enter_context(tc.tile_pool(name="p_xown", bufs=1))
        x_own = p_xown.tile([128, ET, TOWN], F32R)
        p_ctxn = s_ctxn.enter_context(tc.tile_pool(name="p_ctxn", bufs=1))
        ctxn = p_ctxn.tile([128, NP, TOWN], BF16)
        p_z1 = s_z1.enter_context(tc.tile_pool(name="p_z1", bufs=1))
        z1 = p_z1.tile([128, ET, S], BF16)
        p_x = s_x.enter_context(tc.tile_pool(name="p_x", bufs=1))
        x_sb = p_x.tile([128, ET, S], F32R)

        # ---------------- Stage A: LN1 stats + z1 over full sequence -------
        xre = x_t.rearrange("(a p) s -> p a s", p=128)
        CA = 512
        # x chunk DMAs first (so compute never waits), then weight
        # prefetches (which also fix weight-pool slot addresses before any
        # pool is released).
        for c in range(S // CA):
            sl = slice(c * CA, (c + 1) * CA)
            nc.sync.dma_start(out=x_sb[:, :, sl], in_=xre[:, :, sl])
        wv0 = load_wv(0)
        wkq0 = load_wkq(0)
        wd0 = load_w(wout_t, 0, ET)
        wd1 = load_w(wout_t, 1, ET)
        with tc.tile_pool(name="pa_st", bufs=1) as pa_st, \
             tc.tile_pool(name="pa_xsq", bufs=2) as pa_xsq, \
             tc.tile_pool(name="pa_misc", bufs=1) as pa_misc, \
             tc.tile_pool(name="pa_ps", bufs=2, space="PSUM") as pa_ps:
            mean1 = pa_st.tile([128, S], F32)
            rstd1 = pa_st.tile([128, S], F32)
            for c in range(S // CA):
                sl = slice(c * CA, (c + 1) * CA)
                ps_sum = pa_ps.tile([128, CA], F32, tag="ps_sum")
                ps_ssq = pa_ps.tile([128, CA], F32, tag="ps_ssq")
                for a in range(ET):
                    xa = x_sb[:, a, sl]
                    xsq = pa_xsq.tile([128, CA], F32R, tag="xsq")
                    nc.gpsimd.tensor_tensor(xsq, xa, xa, OP.mult)
                    nc.tensor.matmul(ps_sum, ones128, xa,
                                     start=(a == 0), stop=(a == ET - 1))
                    nc.tensor.matmul(ps_ssq, ones128, xsq,
                                     start=(a == 0), stop=(a == ET - 1))
                nc.vector.tensor_scalar_mul(mean1[:, sl], ps_sum, inv_e)
                msq = pa_misc.tile([128, CA], F32, tag="msq")
                nc.vector.scalar_tensor_tensor(
                    msq, mean1[:, sl], unb, mean1[:, sl], OP.mult, OP.mult)
                var = pa_misc.tile([128, CA], F32, tag="var")
                nc.vector.scalar_tensor_tensor(
                    var, ps_ssq, 1.0 / (E - 1.0), msq, OP.mult,
                    OP.subtract)
                std = pa_misc.tile([128, CA], F32, tag="std")
                nc.scalar.activation(std, var, AF.Sqrt)
                nc.vector.tensor_scalar_add(std, std, EPS)
                nc.vector.reciprocal(rstd1[:, sl], std)
                for a in range(ET):
                    nc.vector.tensor_tensor(
                        z1[:, a, sl], x_sb[:, a, sl], mean1[:, sl],
                        OP.subtract)
                    nc.vector.tensor_tensor(
                        z1[:, a, sl], z1[:, a, sl], rstd1[:, sl], OP.mult)
                if c < 2:
                    nc.vector.tensor_copy(x_own[:, :, sl], x_sb[:, :, sl])
        s_x.close()  # x_sb dead after A

        # ------------- Stages B+C interleaved ------------------------------
        # B(p): K/Q projections for head pair p (K over full seq, Q own),
        # V in two 4-pair chunks. C(p): attention for pair p. B(p+2) is
        # emitted before C(p) so the PE has projection matmuls to run while
        # ScalarE computes exp() for pair p.
        p_kqv = s_bc.enter_context(tc.tile_pool(name="p_kqv", bufs=1))
        k_sb = p_kqv.tile([128, NP, S], BF16)
        q_sb = p_kqv.tile([128, NP, TOWN], BF16)
        # [part = t%128, t_tile, pair, head-in-pair, 64 v dims + 1 ones col]
        v_sb = p_kqv.tile([128, NT, NP, 2, 65], BF16)
        nc.vector.tensor_copy(
            v_sb[:, :, :, :, 64],
            ones_f32[:, 0:NT * NP * 2].rearrange(
                "p (a b c) -> p a b c", a=NT, b=NP))
        pb_psv = s_bc.enter_context(
            tc.tile_pool(name="pb_psv", bufs=1, space="PSUM"))
        pb_pskq = s_bc.enter_context(
            tc.tile_pool(name="pb_pskq", bufs=1, space="PSUM"))

        def b_v(cv, wv_c=None):
            if wv_c is None:
                wv_c = load_wv(cv)
            for tt in range(NT):
                tsl = slice(tt * 128, (tt + 1) * 128)
                ps = pb_psv.tile([128, 512], F32, tag="psv")
                for a in range(ET):
                    nc.tensor.matmul(ps, z1[:, a, tsl], wv_c[:, a, :],
                                     start=(a == 0), stop=(a == ET - 1))
                nc.vector.tensor_tensor(
                    v_sb[:, tt, 4 * cv:4 * (cv + 1), :, 0:64],
                    ps.rearrange("p (g hh w) -> p g hh w", g=4, hh=2),
                    vb_bc[:, cv * 512:(cv + 1) * 512].rearrange(
                        "p (g hh w) -> p g hh w", g=4, hh=2),
                    OP.add)

        def b_kq(p, wkq=None):
            wk_p, wq_p = wkq if wkq is not None else load_wkq(p)
            for c in range(4):
                csl = slice(c * 512, (c + 1) * 512)
                ps = pb_pskq.tile([128, 512], F32, tag="pskq")
                for a in range(ET):
                    nc.tensor.matmul(ps, wk_p[:, a, :], z1[:, a, csl],
                                     start=(a == 0), stop=(a == ET - 1))
                nc.vector.tensor_scalar_add(k_sb[:, p, csl], ps,
                                            kb_sb[:, p:p + 1])
            for c in range(2):
                csl = slice(c * 512, (c + 1) * 512)
                ps = pb_pskq.tile([128, 512], F32, tag="pskq")
                for a in range(ET):
                    nc.tensor.matmul(ps, wq_p[:, a, :], z1[:, a, csl],
                                     start=(a == 0), stop=(a == ET - 1))
                nc.vector.tensor_scalar_add(q_sb[:, p, csl], ps,
                                            qb_sb[:, p:p + 1])

        pc_s = s_bc.enter_context(
            tc.tile_pool(name="pc_s", bufs=2, space="PSUM"))
        pc_ctx = s_bc.enter_context(
            tc.tile_pool(name="pc_ctx", bufs=1, space="PSUM"))
        pc_pr = s_bc.enter_context(tc.tile_pool(name="pc_pr", bufs=2))
        pc_m = s_bc.enter_context(tc.tile_pool(name="pc_m", bufs=1))

        def c_attn(p):
            for qc in range(2):
                qsl = slice(qc * 512, (qc + 1) * 512)
                ctx_p = pc_ctx.tile([65, 2, 512], F32, tag="ctx")
                ctx_h = ctx_p[:, 0, :]
                ctx_h2 = ctx_p[:, 1, :]
                for kt in range(NT):
                    k0 = kt * 128
                    s_ps = pc_s.tile([128, 2, 512], F32, tag="s")
                    # 4 packed score MMs: head pair at partitions 0:64 /
                    # 64:128 of k/q, k-token halves at psum 0:64 / 64:128.
                    nc.tensor.matmul(
                        s_ps[0:64, 0, :], k_sb[0:64, p, k0:k0 + 64],
                        q_sb[0:64, p, qsl], start=True, stop=True)
                    nc.tensor.matmul(
                        s_ps[64:128, 0, :], k_sb[0:64, p, k0 + 64:k0 + 128],
                        q_sb[0:64, p, qsl], start=True, stop=True)
                    nc.tensor.matmul(
                        s_ps[0:64, 1, :], k_sb[64:128, p, k0:k0 + 64],
                        q_sb[64:128, p, qsl], start=True, stop=True)
                    nc.tensor.matmul(
                        s_ps[64:128, 1, :],
                        k_sb[64:128, p, k0 + 64:k0 + 128],
                        q_sb[64:128, p, qsl], start=True, stop=True)
                    pr = pc_pr.tile([128, 2, 512], BF16, tag="pr")
                    nc.scalar.activation(pr, s_ps, AF.Exp, scale=0.125)
                    nc.tensor.matmul(ctx_h, v_sb[:, kt, p, 0, :],
                                     pr[:, 0, :],
                                     start=(kt == 0), stop=(kt == NT - 1))
                    nc.tensor.matmul(ctx_h2, v_sb[:, kt, p, 1, :],
                                     pr[:, 1, :],
                                     start=(kt == 0), stop=(kt == NT - 1))
                rec = pc_m.tile([1, 2, 512], BF16, tag="rec")
                with nc.allow_low_precision(
                        reason="bf16 rounding of softmax denom"):
                    nc.vector.reciprocal(rec, ctx_p[64:65, :, :])
                rb_sb = pc_m.tile([64, 2, 512], BF16, tag="rbs")
                nc.gpsimd.partition_broadcast(rb_sb, rec)
                nc.vector.tensor_tensor(
                    ctxn[0:64, p, qsl], ctx_h[0:64, :], rb_sb[:, 0, :],
                    OP.mult)
                nc.vector.tensor_tensor(
                    ctxn[64:128, p, qsl], ctx_h2[0:64, :], rb_sb[:, 1, :],
                    OP.mult)

        b_v(0, wv0)
        b_kq(0, wkq0)
        b_kq(1)
        for p in range(NP):
            if p == 2:
                b_v(1)
            if p + 2 < NP:
                b_kq(p + 2)
            c_attn(p)
        s_bc.close()
        s_z1.close()

        # ------------- Stage D: out-proj + residual ------------------------
        # x2 overwrites x_own in place (the residual add reads x_own[ot,csl]
        # and writes the same region in one DVE instruction).
        x2 = x_own
        with tc.tile_pool(name="pd_ps", bufs=2, space="PSUM") as pdp:
            for c in range(2):
                csl = slice(c * 512, (c + 1) * 512)
                for ot in range(ET):
                    if c == 0 and ot == 0:
                        w_ot = wd0
                    elif c == 0 and ot == 1:
                        w_ot = wd1
                    else:
                        w_ot = load_w(wout_t, ot, ET)
                    ps = pdp.tile([128, 512], F32, tag="ps")
                    for a in range(ET):
                        nc.tensor.matmul(ps, w_ot[:, a, :],
                                         ctxn[:, a, csl],
                                         start=(a == 0), stop=(a == ET - 1))
                    nc.vector.scalar_tensor_tensor(
                        x2[:, ot, csl], ps, ob_sb[:, ot:ot + 1],
                        x_own[:, ot, csl], OP.add, OP.add)
        s_ctxn.close()

        # --------- Stage E: LN2 stats + z2 (own tokens) --------------------
        p_h = s_h.enter_context(tc.tile_pool(name="p_h", bufs=1))
        h_sb = p_h.tile([128, FT, TOWN], BF16)
        p_z2 = s_z2.enter_context(tc.tile_pool(name="p_z2", bufs=1))
        z2 = p_z2.tile([128, ET, TOWN], BF16)
        with tc.tile_pool(name="pe_st", bufs=1) as pe_st, \
             tc.tile_pool(name="pe_tmp", bufs=2) as pe_tmp, \
             tc.tile_pool(name="pe_ps", bufs=2, space="PSUM") as pe_ps:
            mean2 = pe_st.tile([128, TOWN], F32)
            rstd2 = pe_st.tile([128, TOWN], F32)
            for c in range(2):
                sl = slice(c * 512, (c + 1) * 512)
                ps_sum = pe_ps.tile([128, 512], F32, tag="ps_sum")
                ps_ssq = pe_ps.tile([128, 512], F32, tag="ps_ssq")
                for a in range(ET):
                    xa = x2[:, a, sl]
                    xsq = pe_tmp.tile([128, 512], F32R, tag="xsq")
                    nc.vector.tensor_tensor(xsq, xa, xa, OP.mult)
                    nc.tensor.matmul(ps_sum, ones128, xa,
                                     start=(a == 0), stop=(a == ET - 1))
                    nc.tensor.matmul(ps_ssq, ones128, xsq,
                                     start=(a == 0), stop=(a == ET - 1))
                nc.vector.tensor_scalar_mul(mean2[:, sl], ps_sum, inv_e)
                msq = pe_tmp.tile([128, 512], F32, tag="msq")
                nc.vector.scalar_tensor_tensor(
                    msq, mean2[:, sl], unb, mean2[:, sl], OP.mult, OP.mult)
                var = pe_tmp.tile([128, 512], F32, tag="var")
                nc.vector.scalar_tensor_tensor(
                    var, ps_ssq, 1.0 / (E - 1.0), msq, OP.mult, OP.subtract)
                std = pe_tmp.tile([128, 512], F32, tag="std")
                nc.scalar.activation(std, var, AF.Sqrt)
                nc.vector.tensor_scalar_add(std, std, EPS)
                nc.vector.reciprocal(rstd2[:, sl], std)
                for a in range(ET):
                    nc.vector.tensor_tensor(
                        z2[:, a, sl], x2[:, a, sl], mean2[:, sl],
                        OP.subtract)
                    nc.vector.tensor_tensor(
                        z2[:, a, sl], z2[:, a, sl], rstd2[:, sl], OP.mult)

        # --------- Stage F: fc1 + gelu -> h (SBUF) -------------------------
        with tc.tile_pool(name="pf_ps", bufs=2, space="PSUM") as pfp:
            for ft in range(FT):
                w_ft = load_w(wfc1_t, ft, ET)
                for c in range(2):
                    csl = slice(c * 512, (c + 1) * 512)
                    ps = pfp.tile([128, 512], F32, tag="ps")
                    for a in range(ET):
                        nc.tensor.matmul(ps, w_ft[:, a, :], z2[:, a, csl],
                                         start=(a == 0), stop=(a == ET - 1))
                    nc.scalar.activation(h_sb[:, ft, csl], ps, AF.Gelu,
                                         bias=f1b_sb[:, ft:ft + 1])
        s_z2.close()

        # ---------------- Stage G: fc2 + residual -> out_t -----------------
        with tc.tile_pool(name="pg_o", bufs=3) as pgo, \
             tc.tile_pool(name="pg_ps", bufs=2, space="PSUM") as pgp:
            for ot in range(ET):
                w_ot = load_w(wfc2_t, ot, FT)
                for c in range(2):
                    csl = slice(c * 512, (c + 1) * 512)
                    ps = pgp.tile([128, 512], F32, tag="ps")
                    for f in range(FT):
                        nc.tensor.matmul(ps, w_ot[:, f, :], h_sb[:, f, csl],
                                         start=(f == 0), stop=(f == FT - 1))
                    osb = pgo.tile([128, 512], F32, tag="osb")
                    nc.vector.scalar_tensor_tensor(
                        osb, ps, f2b_sb[:, ot:ot + 1], x2[:, ot, csl],
                        OP.add, OP.add)
                    nc.sync.dma_start(
                        out=out_t[ot * 128:(ot + 1) * 128, csl], in_=osb)
        s_h.close()
        s_xown.close()

    nc.finalize()
    return nc


_NC_CACHE = {}


def _get_nc():
    if "nc" not in _NC_CACHE:
        _NC_CACHE["nc"] = _build()
    return _NC_CACHE["nc"]


def _tile_w(w_t, n_out_tiles, dtype=ml_dtypes.bfloat16):
    # [E_in, O] (in-feature rows) -> [O//128, 128, E_in//128, 128] so each
    # output-tile's weight block is contiguous (multi-KB runs per partition).
    e_in, o = w_t.shape
    arr = w_t.reshape(e_in // 128, 128, n_out_tiles, o // n_out_tiles)
    return np.ascontiguousarray(arr.transpose(2, 1, 0, 3).astype(dtype))


def _prepare_in_maps(inputs):
    f = np.float32
    x = np.asarray(inputs["x"], f)
    w_qkv = np.asarray(inputs["w_qkv"], np.float64)
    ln1_w = np.asarray(inputs["ln1_w"], np.float64)
    ln1_b = np.asarray(inputs["ln1_b"], np.float64)
    ln2_w = np.asarray(inputs["ln2_w"], np.float64)
    ln2_b = np.asarray(inputs["ln2_b"], np.float64)
    w_fc1 = np.asarray(inputs["w_fc1"], np.float64)

    wqkv_s = (w_qkv * ln1_w[None, :])  # fold LN1 gamma
    qkv_bias = ln1_b @ np.asarray(inputs["w_qkv"], np.float64).T  # [3E]
    wqkv_t = np.ascontiguousarray(wqkv_s.T, f)  # [E, 3E]
    wq_t = _tile_w(wqkv_t[:, 0:E], ET)
    wk_t = _tile_w(wqkv_t[:, E:2 * E], ET)
    wv_t = _tile_w(wqkv_t[:, 2 * E:3 * E], 2)  # [2,128,ET,512] rhs chunks
    col = lambda v: np.ascontiguousarray(
        np.asarray(v, f).reshape(-1, 128).T)  # [o] -> [128, o//128]
    qb = col(qkv_bias[0:E])
    kb = col(qkv_bias[E:2 * E])
    vb = np.ascontiguousarray(qkv_bias[2 * E:3 * E]).astype(ml_dtypes.bfloat16)

    wout_t = _tile_w(np.ascontiguousarray(np.asarray(inputs["w_out"], f).T),
                     ET)
    ob = col(inputs["b_out"])

    wfc1_s = (w_fc1 * ln2_w[None, :])
    f1b_flat = np.asarray(inputs["b_fc1"], np.float64) + ln2_b @ w_fc1.T
    f1b = col(f1b_flat)
    wfc1_t = _tile_w(np.ascontiguousarray(wfc1_s.T, f), FT)
    wfc2_t = _tile_w(np.ascontiguousarray(np.asarray(inputs["w_fc2"], f).T),
                     ET)
    f2b = col(inputs["b_fc2"])

    shared = dict(wq_t=wq_t, wk_t=wk_t, wv_t=wv_t, qb=qb, kb=kb, vb=vb,
                  wout_t=wout_t, ob=ob, wfc1_t=wfc1_t, f1b=f1b,
                  wfc2_t=wfc2_t, f2b=f2b)
    in_maps = []
    for core in range(NCORES):
        b, hf = divmod(core, 2)
        xs = np.roll(x[b], -hf * TOWN, axis=0)  # own tokens first
        x_tc = np.ascontiguousarray(xs.T)  # [E, S]
        in_maps.append(dict(x_t=x_tc, **shared))
    return in_maps


def _assemble(inputs, results):
    out = np.empty((B, S, E), np.float32)
    for core in range(NCORES):
        b, hf = divmod(core, 2)
        out[b, hf * TOWN:(hf + 1) * TOWN, :] = results[core]["out_t"].T
    return out


def run(inputs, **spmd_kwargs):
    nc = _get_nc()
    in_maps = _prepare_in_maps(inputs)
    res = run_bass_kernel_spmd(nc, in_maps, core_ids=list(range(NCORES)),
                               **spmd_kwargs)
    return _assemble(inputs, res.results), res


def kernel(**inputs):
    out, _ = run(inputs)
    return out


# revision 11
# speedup vs baseline: 1.3807x; 1.1465x over previous
"""Encoder layer (pre-norm attention + MLP) on 8 Trainium2 cores.

Sharding: core = (batch b in 0..3, half hf in 0..1). Each core receives the
full 2048-token sequence of batch b, transposed to [E, S] and rolled so the
core's own 1024 tokens are columns 0:1024 (attention and LN are invariant to
key order, so rolling keeps the program identical across cores). The core
computes K/V over the full sequence and everything else only for its own
tokens. No collectives; the host reassembles the 8 shards.

v2: everything stays in SBUF (K/Q/V, the MLP hidden h) — no DRAM round
trips. fc1/fc2 weights and activations are bf16 (halves weight DMA).
Attention scores are packed 4 MMs per PE pass via base-partition tile
placement (head pairs live at partitions 0:64 / 64:128), recovering the
full 128x128 array despite D=64. QKV projection matmuls are emitted
interleaved with attention so the PE has filler work while ScalarE runs
the softmax exponentials. The final output fuses fc2 + b_fc2 + attention
residual on-chip; the host only transposes.
"""

import numpy as np
import ml_dtypes
from contextlib import ExitStack

import concourse.bacc as bacc
import concourse.mybir as mybir
import concourse.tile as tile
from concourse.bass_utils import run_bass_kernel_spmd

F32 = mybir.dt.float32
F32R = mybir.dt.float32r
BF16 = mybir.dt.bfloat16
AF = mybir.ActivationFunctionType
OP = mybir.AluOpType

B, S, E, H, D, FF = 4, 2048, 1024, 16, 64, 4096
TOWN = 1024  # tokens owned per core
ET = E // 128  # 8
FT = FF // 128  # 32
NT = S // 128  # 16 token tiles (full seq)
NP = H // 2  # 8 head pairs
NCORES = 8
EPS = 1e-6


def _build():
    nc = bacc.Bacc()

    x_t = nc.dram_tensor("x_t", [E, S], F32R, kind="ExternalInput")
    # weights pre-tiled on host: [out_tile, 128(part=e%128), e_tile, out_in_tile]
    wq_t = nc.dram_tensor("wq_t", [ET, 128, ET, 128], BF16,
                          kind="ExternalInput")
    wk_t = nc.dram_tensor("wk_t", [ET, 128, ET, 128], BF16,
                          kind="ExternalInput")
    wv_t = nc.dram_tensor("wv_t", [2, 128, ET, 512], BF16,
                          kind="ExternalInput")
    qb = nc.dram_tensor("qb", [128, ET], F32, kind="ExternalInput")
    kb = nc.dram_tensor("kb", [128, ET], F32, kind="ExternalInput")
    vb = nc.dram_tensor("vb", [E], BF16, kind="ExternalInput")
    wout_t = nc.dram_tensor("wout_t", [ET, 128, ET, 128], BF16,
                            kind="ExternalInput")
    ob = nc.dram_tensor("ob", [128, ET], F32, kind="ExternalInput")
    wfc1_t = nc.dram_tensor("wfc1_t", [FT, 128, ET, 128], BF16,
                            kind="ExternalInput")
    f1b = nc.dram_tensor("f1b", [128, FT], F32, kind="ExternalInput")
    wfc2_t = nc.dram_tensor("wfc2_t", [ET, 128, FT, 128], BF16,
                            kind="ExternalInput")
    f2b = nc.dram_tensor("f2b", [128, ET], F32, kind="ExternalInput")

    out_t = nc.dram_tensor("out_t", [E, TOWN], F32, kind="ExternalOutput")

    inv_e = 1.0 / E
    unb = float(E) / (E - 1.0)  # E/(E-1) for unbiased variance

    with tile.TileContext(nc) as tc, ExitStack() as ctx:
        consts = ctx.enter_context(tc.tile_pool(name="consts", bufs=1))
        ones_f32 = consts.tile([128, 256], F32)
        nc.vector.memset(ones_f32, 1.0)
        ones128 = consts.tile([128, 128], F32R)
        nc.vector.tensor_copy(ones128, ones_f32[:, 0:128])
        qb_sb = consts.tile([128, ET], F32)
        kb_sb = consts.tile([128, ET], F32)
        ob_sb = consts.tile([128, ET], F32)
        f1b_sb = consts.tile([128, FT], F32)
        f2b_sb = consts.tile([128, ET], F32)
        nc.sync.dma_start(out=qb_sb, in_=qb[:, :])
        nc.sync.dma_start(out=kb_sb, in_=kb[:, :])
        nc.sync.dma_start(out=ob_sb, in_=ob[:, :])
        nc.sync.dma_start(out=f1b_sb, in_=f1b[:, :])
        nc.sync.dma_start(out=f2b_sb, in_=f2b[:, :])
        ones_bf16 = consts.tile([1, 128], BF16)
        nc.vector.tensor_copy(ones_bf16, ones_f32[0:1, 0:128])
        # v bias broadcast across all partitions (v is token-major)
        vb_row = consts.tile([1, E], BF16)
        nc.sync.dma_start(out=vb_row, in_=vb[None, :])
        vb_bc = consts.tile([128, E], BF16)
        with tc.tile_pool(name="vbbc_p", bufs=2, space="PSUM") as vbbc_p:
            for c in range(2):
                ps = vbbc_p.tile([128, 512], F32, tag="vbbc")
                nc.tensor.matmul(
                    ps, ones_bf16,
                    vb_row[:, c * 512:(c + 1) * 512],
                    start=True, stop=True,
                )
                nc.vector.tensor_copy(vb_bc[:, c * 512:(c + 1) * 512], ps)

        # Weight pools live in the outer scope (never released): tiles
        # written by input DMA must not reuse released pool space (the DMA
        # has no happens-before chain to prior readers). All slots are
        # materialized up front by prefetching the first weight tiles.
        pb_wv = ctx.enter_context(tc.tile_pool(name="pb_wv", bufs=1))
        pb_w = ctx.enter_context(tc.tile_pool(name="pb_w", bufs=2))
        pw = ctx.enter_context(tc.tile_pool(name="pw", bufs=2))

        def load_wv(cv):
            wv_c = pb_wv.tile([128, ET, 512], BF16, tag="wv")
            nc.sync.dma_start(out=wv_c, in_=wv_t[cv])
            return wv_c

        def load_wkq(p):
            wk_p = pb_w.tile([128, ET, 128], BF16, tag="wk")
            nc.sync.dma_start(out=wk_p, in_=wk_t[p])
            wq_p = pb_w.tile([128, ET, 128], BF16, tag="wk")
            nc.sync.dma_start(out=wq_p, in_=wq_t[p])
            return wk_p, wq_p

        def load_w(src, ot, n_in):
            w = pw.tile([128, n_in, 128], BF16, tag="w")
            nc.sync.dma_start(out=w, in_=src[ot])
            return w


        # Long-lived cross-stage tensors. Pools are opened/closed in LIFO
        # (stack) order; x2 reuses the x_own buffer in place after D.
        s_xown = ExitStack()   # x_own: A..D, then reused as x2: D..G
        s_ctxn = ExitStack()   # ctxn: C..D
        s_z1 = ExitStack()     # z1: A..C (needed through B emissions)
        s_bc = ExitStack()     # k/q/v + B/C working pools: B..C
        s_x = ExitStack()      # x_sb: A only
        s_z2 = ExitStack()     # z2: E..F
        s_h = ExitStack()      # h: F..G

        p_xown = s_xown.enter_context(tc.tile_pool(name="p_xown", bufs=1))
        x_own = p_xown.tile([128, ET, TOWN], F32R)
        p_ctxn = s_ctxn.enter_context(tc.tile_pool(name="p_ctxn", bufs=1))
        ctxn = p_ctxn.tile([128, NP, TOWN], BF16)
        p_z1 = s_z1.enter_context(tc.tile_pool(name="p_z1", bufs=1))
        z1 = p_z1.tile([128, ET, S], BF16)
        p_x = s_x.enter_context(tc.tile_pool(name="p_x", bufs=1))
        x_sb = p_x.tile([128, ET, S], F32R)

        # ---------------- Stage A: LN1 stats + z1 over full sequence -------
        xre = x_t.rearrange("(a p) s -> p a s", p=128)
        CA = 512
        # x chunk DMAs first (so compute never waits), then weight
        # prefetches (which also fix weight-pool slot addresses before any
        # pool is released).
        for c in range(S // CA):
            sl = slice(c * CA, (c + 1) * CA)
            nc.sync.dma_start(out=x_sb[:, :, sl], in_=xre[:, :, sl])
        wv0 = load_wv(0)
        wkq0 = load_wkq(0)
        wd0 = load_w(wout_t, 0, ET)
        wd1 = load_w(wout_t, 1, ET)
        with tc.tile_pool(name="pa_st", bufs=1) as pa_st, \
             tc.tile_pool(name="pa_xsq", bufs=2) as pa_xsq, \
             tc.tile_pool(name="pa_misc", bufs=1) as pa_misc, \
             tc.tile_pool(name="pa_ps", bufs=2, space="PSUM") as pa_ps:
            mean1 = pa_st.tile([128, S], F32)
            rstd1 = pa_st.tile([128, S], F32)
            for c in range(S // CA):
                sl = slice(c * CA, (c + 1) * CA)
                ps_sum = pa_ps.tile([128, CA], F32, tag="ps_sum")
                ps_ssq = pa_ps.tile([128, CA], F32, tag="ps_ssq")
                for a in range(ET):
                    xa = x_sb[:, a, sl]
                    xsq = pa_xsq.tile([128, CA], F32R, tag="xsq")
                    nc.vector.tensor_tensor(xsq, xa, xa, OP.mult)
                    nc.tensor.matmul(ps_sum, ones128, xa,
                                     start=(a == 0), stop=(a == ET - 1))
                    nc.tensor.matmul(ps_ssq, ones128, xsq,
                                     start=(a == 0), stop=(a == ET - 1))
                nc.vector.tensor_scalar_mul(mean1[:, sl], ps_sum, inv_e)
                msq = pa_misc.tile([128, CA], F32, tag="msq")
                nc.vector.scalar_tensor_tensor(
                    msq, mean1[:, sl], unb, mean1[:, sl], OP.mult, OP.mult)
                var = pa_misc.tile([128, CA], F32, tag="var")
                nc.vector.scalar_tensor_tensor(
                    var, ps_ssq, 1.0 / (E - 1.0), msq, OP.mult,
                    OP.subtract)
                std = pa_misc.tile([128, CA], F32, tag="std")
                nc.scalar.activation(std, var, AF.Sqrt)
                nc.vector.tensor_scalar_add(std, std, EPS)
                nc.vector.reciprocal(rstd1[:, sl], std)
                for a in range(ET):
                    nc.vector.tensor_tensor(
                        z1[:, a, sl], x_sb[:, a, sl], mean1[:, sl],
                        OP.subtract)
                    nc.vector.tensor_tensor(
                        z1[:, a, sl], z1[:, a, sl], rstd1[:, sl], OP.mult)
                if c < 2:
                    nc.vector.tensor_copy(x_own[:, :, sl], x_sb[:, :, sl])
        s_x.close()  # x_sb dead after A

        # ------------- Stages B+C interleaved ------------------------------
        # B(p): K/Q projections for head pair p (K over full seq, Q own),
        # V in two 4-pair chunks. C(p): attention for pair p. B(p+2) is
        # emitted before C(p) so the PE has projection matmuls to run while
        # ScalarE computes exp() for pair p.
        p_kqv = s_bc.enter_context(tc.tile_pool(name="p_kqv", bufs=1))
        k_sb = p_kqv.tile([128, NP, S], BF16)
        q_sb = p_kqv.tile([128, NP, TOWN], BF16)
        # [part = t%128, t_tile, pair, head-in-pair, 64 v dims + 1 ones col]
        v_sb = p_kqv.tile([128, NT, NP, 2, 65], BF16)
        nc.vector.tensor_copy(
            v_sb[:, :, :, :, 64],
            ones_f32[:, 0:NT * NP * 2].rearrange(
                "p (a b c) -> p a b c", a=NT, b=NP))
        pb_psv = s_bc.enter_context(
            tc.tile_pool(name="pb_psv", bufs=1, space="PSUM"))
        pb_pskq = s_bc.enter_context(
            tc.tile_pool(name="pb_pskq", bufs=1, space="PSUM"))

        def b_v(cv, wv_c=None):
            if wv_c is None:
                wv_c = load_wv(cv)
            for tt in range(NT):
                tsl = slice(tt * 128, (tt + 1) * 128)
                ps = pb_psv.tile([128, 512], F32, tag="psv")
                for a in range(ET):
                    nc.tensor.matmul(ps, z1[:, a, tsl], wv_c[:, a, :],
                                     start=(a == 0), stop=(a == ET - 1))
                nc.vector.tensor_tensor(
                    v_sb[:, tt, 4 * cv:4 * (cv + 1), :, 0:64],
                    ps.rearrange("p (g hh w) -> p g hh w", g=4, hh=2),
                    vb_bc[:, cv * 512:(cv + 1) * 512].rearrange(
                        "p (g hh w) -> p g hh w", g=4, hh=2),
                    OP.add)

        def b_kq(p, wkq=None):
            wk_p, wq_p = wkq if wkq is not None else load_wkq(p)
            for c in range(4):
                csl = slice(c * 512, (c + 1) * 512)
                ps = pb_pskq.tile([128, 512], F32, tag="pskq")
                for a in range(ET):
                    nc.tensor.matmul(ps, wk_p[:, a, :], z1[:, a, csl],
                                     start=(a == 0), stop=(a == ET - 1))
                nc.vector.tensor_scalar_add(k_sb[:, p, csl], ps,
                                            kb_sb[:, p:p + 1])
            for c in range(2):
                csl = slice(c * 512, (c + 1) * 512)
                ps = pb_pskq.tile([128, 512], F32, tag="pskq")
                for a in range(ET):
                    nc.tensor.matmul(ps, wq_p[:, a, :], z1[:, a, csl],
                                     start=(a == 0), stop=(a == ET - 1))
                nc.vector.tensor_scalar_add(q_sb[:, p, csl], ps,
                                            qb_sb[:, p:p + 1])

        pc_s = s_bc.enter_context(
            tc.tile_pool(name="pc_s", bufs=2, space="PSUM"))
        pc_ctx = s_bc.enter_context(
            tc.tile_pool(name="pc_ctx", bufs=1, space="PSUM"))
        pc_pr = s_bc.enter_context(tc.tile_pool(name="pc_pr", bufs=2))
        pc_m = s_bc.enter_context(tc.tile_pool(name="pc_m", bufs=1))

        def c_attn_qc(p, qc):
            qsl = slice(qc * 512, (qc + 1) * 512)
            ctx_p = pc_ctx.tile([65, 2, 512], F32, tag="ctx")
            prs = [None] * NT

            def ctx_mms(kt):
                nc.tensor.matmul(ctx_p[:, 0, :], v_sb[:, kt, p, 0, :],
                                 prs[kt][:, 0, :],
                                 start=(kt == 0), stop=(kt == NT - 1))
                nc.tensor.matmul(ctx_p[:, 1, :], v_sb[:, kt, p, 1, :],
                                 prs[kt][:, 1, :],
                                 start=(kt == 0), stop=(kt == NT - 1))

            for kt in range(NT):
                k0 = kt * 128
                s_ps = pc_s.tile([128, 2, 512], F32, tag="s")
                # 4 packed score MMs: head pair at partitions 0:64 /
                # 64:128 of k/q, k-token halves at psum 0:64 / 64:128.
                nc.tensor.matmul(
                    s_ps[0:64, 0, :], k_sb[0:64, p, k0:k0 + 64],
                    q_sb[0:64, p, qsl], start=True, stop=True)
                nc.tensor.matmul(
                    s_ps[64:128, 0, :], k_sb[0:64, p, k0 + 64:k0 + 128],
                    q_sb[0:64, p, qsl], start=True, stop=True)
                nc.tensor.matmul(
                    s_ps[0:64, 1, :], k_sb[64:128, p, k0:k0 + 64],
                    q_sb[64:128, p, qsl], start=True, stop=True)
                nc.tensor.matmul(
                    s_ps[64:128, 1, :],
                    k_sb[64:128, p, k0 + 64:k0 + 128],
                    q_sb[64:128, p, qsl], start=True, stop=True)
                pr = pc_pr.tile([128, 2, 512], BF16, tag="pr")
                nc.scalar.activation(pr, s_ps, AF.Exp, scale=0.125)
                prs[kt] = pr
                # ctx matmuls lag one kt so the PE never sits behind a
                # matmul that waits on the exp it just requested.
                if kt >= 1:
                    ctx_mms(kt - 1)
            ctx_mms(NT - 1)
            # Tail, decoupled from the PSUM accumulator: copy ctx to SBUF
            # and take the reciprocal of the denominator row; the psum slot
            # frees as soon as both reads retire.
            ctx_sb = pc_m.tile([65, 2, 512], BF16, tag="ctxsb")
            nc.vector.tensor_copy(ctx_sb, ctx_p)
            rec = pc_m.tile([1, 2, 512], BF16, tag="rec")
            with nc.allow_low_precision(
                    reason="bf16 rounding of softmax denom"):
                nc.vector.reciprocal(rec, ctx_sb[64:65, :, :])
            rb_sb = pc_m.tile([64, 2, 512], BF16, tag="rbs")
            nc.gpsimd.partition_broadcast(rb_sb, rec)
            nc.vector.tensor_tensor(
                ctxn[0:64, p, qsl], ctx_sb[0:64, 0, :], rb_sb[:, 0, :],
                OP.mult)
            nc.vector.tensor_tensor(
                ctxn[64:128, p, qsl], ctx_sb[0:64, 1, :], rb_sb[:, 1, :],
                OP.mult)

        b_v(0, wv0)
        b_kq(0, wkq0)
        b_kq(1)
        for p in range(NP):
            if p == 2:
                b_v(1)
            c_attn_qc(p, 0)
            if p + 2 < NP:
                b_kq(p + 2)
            c_attn_qc(p, 1)
        s_bc.close()
        s_z1.close()

        # ------------- Stage D: out-proj + residual ------------------------
        # x2 overwrites x_own in place (the residual add reads x_own[ot,csl]
        # and writes the same region in one DVE instruction).
        x2 = x_own
        with tc.tile_pool(name="pd_ps", bufs=2, space="PSUM") as pdp:
            for c in range(2):
                csl = slice(c * 512, (c + 1) * 512)
                for ot in range(ET):
                    if c == 0 and ot == 0:
                        w_ot = wd0
                    elif c == 0 and ot == 1:
                        w_ot = wd1
                    else:
                        w_ot = load_w(wout_t, ot, ET)
                    ps = pdp.tile([128, 512], F32, tag="ps")
                    for a in range(ET):
                        nc.tensor.matmul(ps, w_ot[:, a, :],
                                         ctxn[:, a, csl],
                                         start=(a == 0), stop=(a == ET - 1))
                    nc.vector.scalar_tensor_tensor(
                        x2[:, ot, csl], ps, ob_sb[:, ot:ot + 1],
                        x_own[:, ot, csl], OP.add, OP.add)
        s_ctxn.close()

        # --------- Stage E: LN2 stats + z2 (own tokens) --------------------
        p_h = s_h.enter_context(tc.tile_pool(name="p_h", bufs=1))
        h_sb = p_h.tile([128, FT, TOWN], BF16)
        p_z2 = s_z2.enter_context(tc.tile_pool(name="p_z2", bufs=1))
        z2 = p_z2.tile([128, ET, TOWN], BF16)
        with tc.tile_pool(name="pe_st", bufs=1) as pe_st, \
             tc.tile_pool(name="pe_tmp", bufs=2) as pe_tmp, \
             tc.tile_pool(name="pe_ps", bufs=2, space="PSUM") as pe_ps:
            mean2 = pe_st.tile([128, TOWN], F32)
            rstd2 = pe_st.tile([128, TOWN], F32)
            for c in range(2):
                sl = slice(c * 512, (c + 1) * 512)
                ps_sum = pe_ps.tile([128, 512], F32, tag="ps_sum")
                ps_ssq = pe_ps.tile([128, 512], F32, tag="ps_ssq")
                for a in range(ET):
                    xa = x2[:, a, sl]
                    xsq = pe_tmp.tile([128, 512], F32R, tag="xsq")
                    nc.vector.tensor_tensor(xsq, xa, xa, OP.mult)
                    nc.tensor.matmul(ps_sum, ones128, xa,
                                     start=(a == 0), stop=(a == ET - 1))
                    nc.tensor.matmul(ps_ssq, ones128, xsq,
                                     start=(a == 0), stop=(a == ET - 1))
                nc.vector.tensor_scalar_mul(mean2[:, sl], ps_sum, inv_e)
                msq = pe_tmp.tile([128, 512], F32, tag="msq")
                nc.vector.scalar_tensor_tensor(
                    msq, mean2[:, sl], unb, mean2[:, sl], OP.mult, OP.mult)
                var = pe_tmp.tile([128, 512], F32, tag="var")
                nc.vector.scalar_tensor_tensor(
                    var, ps_ssq, 1.0 / (E - 1.0), msq, OP.mult, OP.subtract)
                std = pe_tmp.tile([128, 512], F32, tag="std")
                nc.scalar.activation(std, var, AF.Sqrt)
                nc.vector.tensor_scalar_add(std, std, EPS)
                nc.vector.reciprocal(rstd2[:, sl], std)
                for a in range(ET):
                    nc.vector.tensor_tensor(
                        z2[:, a, sl], x2[:, a, sl], mean2[:, sl],
                        OP.subtract)
                    nc.vector.tensor_tensor(
                        z2[:, a, sl], z2[:, a, sl], rstd2[:, sl], OP.mult)

        # --------- Stage F: fc1 + gelu -> h (SBUF) -------------------------
        with tc.tile_pool(name="pf_ps", bufs=2, space="PSUM") as pfp:
            for ft in range(FT):
                w_ft = load_w(wfc1_t, ft, ET)
                for c in range(2):
                    csl = slice(c * 512, (c + 1) * 512)
                    ps = pfp.tile([128, 512], F32, tag="ps")
                    for a in range(ET):
                        nc.tensor.matmul(ps, w_ft[:, a, :], z2[:, a, csl],
                                         start=(a == 0), stop=(a == ET - 1))
                    nc.scalar.activation(h_sb[:, ft, csl], ps, AF.Gelu,
                                         bias=f1b_sb[:, ft:ft + 1])
        s_z2.close()

        # ---------------- Stage G: fc2 + residual -> out_t -----------------
        with tc.tile_pool(name="pg_o", bufs=3) as pgo, \
             tc.tile_pool(name="pg_ps", bufs=2, space="PSUM") as pgp:
            for ot in range(ET):
                w_lo = pw.tile([128, FT // 2, 128], BF16, tag="w")
                nc.sync.dma_start(out=w_lo, in_=wfc2_t[ot, :, 0:FT // 2])
                w_hi = pw.tile([128, FT // 2, 128], BF16, tag="w")
                nc.sync.dma_start(out=w_hi, in_=wfc2_t[ot, :, FT // 2:FT])
                for c in range(2):
                    csl = slice(c * 512, (c + 1) * 512)
                    ps = pgp.tile([128, 512], F32, tag="ps")
                    for f in range(FT):
                        w_f = w_lo[:, f, :] if f < FT // 2 else \
                            w_hi[:, f - FT // 2, :]
                        nc.tensor.matmul(ps, w_f, h_sb[:, f, csl],
                                         start=(f == 0), stop=(f == FT - 1))
                    osb = pgo.tile([128, 512], F32, tag="osb")
                    nc.vector.scalar_tensor_tensor(
                        osb, ps, f2b_sb[:, ot:ot + 1], x2[:, ot, csl],
                        OP.add, OP.add)
                    nc.sync.dma_start(
                        out=out_t[ot * 128:(ot + 1) * 128, csl], in_=osb)
        s_h.close()
        s_xown.close()

    nc.finalize()
    return nc


_NC_CACHE = {}


def _get_nc():
    if "nc" not in _NC_CACHE:
        _NC_CACHE["nc"] = _build()
    return _NC_CACHE["nc"]


def _tile_w(w_t, n_out_tiles, dtype=ml_dtypes.bfloat16):
    # [E_in, O] (in-feature rows) -> [O//128, 128, E_in//128, 128] so each
    # output-tile's weight block is contiguous (multi-KB runs per partition).
    e_in, o = w_t.shape
    arr = w_t.reshape(e_in // 128, 128, n_out_tiles, o // n_out_tiles)
    return np.ascontiguousarray(arr.transpose(2, 1, 0, 3).astype(dtype))


def _prepare_in_maps(inputs):
    f = np.float32
    x = np.asarray(inputs["x"], f)
    w_qkv = np.asarray(inputs["w_qkv"], np.float64)
    ln1_w = np.asarray(inputs["ln1_w"], np.float64)
    ln1_b = np.asarray(inputs["ln1_b"], np.float64)
    ln2_w = np.asarray(inputs["ln2_w"], np.float64)
    ln2_b = np.asarray(inputs["ln2_b"], np.float64)
    w_fc1 = np.asarray(inputs["w_fc1"], np.float64)

    wqkv_s = (w_qkv * ln1_w[None, :])  # fold LN1 gamma
    qkv_bias = ln1_b @ np.asarray(inputs["w_qkv"], np.float64).T  # [3E]
    wqkv_t = np.ascontiguousarray(wqkv_s.T, f)  # [E, 3E]
    wq_t = _tile_w(wqkv_t[:, 0:E], ET)
    wk_t = _tile_w(wqkv_t[:, E:2 * E], ET)
    wv_t = _tile_w(wqkv_t[:, 2 * E:3 * E], 2)  # [2,128,ET,512] rhs chunks
    col = lambda v: np.ascontiguousarray(
        np.asarray(v, f).reshape(-1, 128).T)  # [o] -> [128, o//128]
    qb = col(qkv_bias[0:E])
    kb = col(qkv_bias[E:2 * E])
    vb = np.ascontiguousarray(qkv_bias[2 * E:3 * E]).astype(ml_dtypes.bfloat16)

    wout_t = _tile_w(np.ascontiguousarray(np.asarray(inputs["w_out"], f).T),
                     ET)
    ob = col(inputs["b_out"])

    wfc1_s = (w_fc1 * ln2_w[None, :])
    f1b_flat = np.asarray(inputs["b_fc1"], np.float64) + ln2_b @ w_fc1.T
    f1b = col(f1b_flat)
    wfc1_t = _tile_w(np.ascontiguousarray(wfc1_s.T, f), FT)
    wfc2_t = _tile_w(np.ascontiguousarray(np.asarray(inputs["w_fc2"], f).T),
                     ET)
    f2b = col(inputs["b_fc2"])

    shared = dict(wq_t=wq_t, wk_t=wk_t, wv_t=wv_t, qb=qb, kb=kb, vb=vb,
                  wout_t=wout_t, ob=ob, wfc1_t=wfc1_t, f1b=f1b,
                  wfc2_t=wfc2_t, f2b=f2b)
    in_maps = []
    for core in range(NCORES):
        b, hf = divmod(core, 2)
        xs = np.roll(x[b], -hf * TOWN, axis=0)  # own tokens first
        x_tc = np.ascontiguousarray(xs.T)  # [E, S]
        in_maps.append(dict(x_t=x_tc, **shared))
    return in_maps


def _assemble(inputs, results):
    out = np.empty((B, S, E), np.float32)
    for core in range(NCORES):
        b, hf = divmod(core, 2)
        out[b, hf * TOWN:(hf + 1) * TOWN, :] = results[core]["out_t"].T
    return out


def run(inputs, **spmd_kwargs):
    nc = _get_nc()
    in_maps = _prepare_in_maps(inputs)
    res = run_bass_kernel_spmd(nc, in_maps, core_ids=list(range(NCORES)),
                               **spmd_kwargs)
    return _assemble(inputs, res.results), res


def kernel(**inputs):
    out, _ = run(inputs)
    return out
